# revision 6
# baseline (speedup 1.0000x reference)
import numpy as np

import concourse.bacc as bacc
import concourse.mybir as mybir
import concourse.tile as tile
from concourse.bass_utils import run_bass_kernel_spmd

B, NIN, NH, NOUT = 4096, 2048, 1024, 512
NCORES = 8
BS = B // NCORES          # 512 batch rows per core
STEPS = 5
BETA = 0.95
THR = 1.0
KC1 = NIN // 128          # 16 k-chunks for fc1
MC1 = NH // 128           # 8 output chunks for fc1
KC2 = NH // 128           # 8 k-chunks for fc2
MC2 = NOUT // 128         # 4 output chunks for fc2
F16 = mybir.dt.float16
F32 = mybir.dt.float32

_prog = None


def _build_program():
    nc = bacc.Bacc(None, target_bir_lowering=False, debug=False)

    d_xh = nc.dram_tensor("x_hi", [NIN, BS], F16, kind="ExternalInput")
    d_xl = nc.dram_tensor("x_lo", [NIN, BS], F16, kind="ExternalInput")
    d_w1h = nc.dram_tensor("w1_hi", [NIN, NH], F16, kind="ExternalInput")
    d_w1l = nc.dram_tensor("w1_lo", [NIN, NH], F16, kind="ExternalInput")
    d_w2h = nc.dram_tensor("w2_hi", [NH, NOUT], F16, kind="ExternalInput")
    d_w2l = nc.dram_tensor("w2_lo", [NH, NOUT], F16, kind="ExternalInput")
    d_b1 = nc.dram_tensor("b1", [NH, 1], F32, kind="ExternalInput")
    d_b2h = nc.dram_tensor("b2_hi", [1, NOUT], F16, kind="ExternalInput")
    d_b2l = nc.dram_tensor("b2_lo", [1, NOUT], F16, kind="ExternalInput")
    d_negI = nc.dram_tensor("negI", [128, 128], F16, kind="ExternalInput")

    d_spk = nc.dram_tensor("spk_out", [STEPS, 128, MC2 * BS], F16, kind="ExternalOutput")
    d_mem = nc.dram_tensor("mem_out", [STEPS, 128, MC2 * BS], F32, kind="ExternalOutput")

    AF = mybir.ActivationFunctionType
    OP = mybir.AluOpType

    with tile.TileContext(nc) as tc:
        with (
            tc.tile_pool(name="sb", bufs=1) as pool,
            tc.tile_pool(name="ps", bufs=1, space="PSUM") as psum,
        ):
            def P(name, shape, dt):
                return pool.tile(shape, dt, name=name, tag=name, bufs=1)

            def bank(i, name):
                return psum.tile([128, BS], F32, name=name, tag=f"bank{i}", bufs=1)

            xh = P("xh", [128, KC1 * BS], F16)
            xl = P("xl", [128, KC1 * BS], F16)
            w1h = P("w1h", [128, KC1 * NH], F16)
            w1l = P("w1l", [128, KC1 * NH], F16)
            w2h = P("w2h", [128, KC2 * NOUT], F16)
            w2l = P("w2l", [128, KC2 * NOUT], F16)
            b1sb = P("b1sb", [128, MC1], F32)
            b2h = P("b2h", [1, NOUT], F16)
            b2l = P("b2l", [1, NOUT], F16)
            negI = P("negI_sb", [128, 128], F16)
            ones = P("ones", [1, BS], F16)
            cur1 = P("cur1", [128, MC1 * BS], F32)
            mem1 = P("mem1", [128, MC1 * BS], F32)
            spk1 = P("spk1", [128, MC1 * BS], F16)

            nc.gpsimd.memset(ones[:], 1.0)

            # ---- input DMAs ----
            nc.sync.dma_start(
                out=xh[:].rearrange("p (k b) -> p k b", k=KC1),
                in_=d_xh[:].rearrange("(k p) b -> p k b", p=128),
            )
            nc.sync.dma_start(
                out=xl[:].rearrange("p (k b) -> p k b", k=KC1),
                in_=d_xl[:].rearrange("(k p) b -> p k b", p=128),
            )
            nc.sync.dma_start(
                out=w1h[:].rearrange("p (k n) -> p k n", k=KC1),
                in_=d_w1h[:].rearrange("(k p) n -> p k n", p=128),
            )
            nc.sync.dma_start(
                out=w1l[:].rearrange("p (k n) -> p k n", k=KC1),
                in_=d_w1l[:].rearrange("(k p) n -> p k n", p=128),
            )
            nc.sync.dma_start(
                out=w2h[:].rearrange("p (k n) -> p k n", k=KC2),
                in_=d_w2h[:].rearrange("(k p) n -> p k n", p=128),
            )
            nc.sync.dma_start(
                out=w2l[:].rearrange("p (k n) -> p k n", k=KC2),
                in_=d_w2l[:].rearrange("(k p) n -> p k n", p=128),
            )
            nc.sync.dma_start(
                out=b1sb[:].rearrange("p (m one) -> p m one", one=1),
                in_=d_b1[:].rearrange("(m p) one -> p m one", p=128),
            )
            nc.sync.dma_start(out=b2h[:], in_=d_b2h[:])
            nc.sync.dma_start(out=b2l[:], in_=d_b2l[:])
            nc.sync.dma_start(out=negI[:], in_=d_negI[:])

            # ---- fc1: cur1[nh, b] = W1.T(k, nh)^T x.T(k, b), hi/lo 3-pass ----
            ps1 = [bank(m, f"ps1_{m}") for m in range(MC1)]
            for k in range(KC1):
                xh_k = xh[:, k * BS:(k + 1) * BS]
                xl_k = xl[:, k * BS:(k + 1) * BS]
                for m in range(MC1):
                    w_off = k * NH + m * 128
                    wh_km = w1h[:, w_off:w_off + 128]
                    wl_km = w1l[:, w_off:w_off + 128]
                    nc.tensor.matmul(
                        out=ps1[m][:], lhsT=wh_km, rhs=xh_k,
                        start=(k == 0), stop=False,
                    )
                    nc.tensor.matmul(
                        out=ps1[m][:], lhsT=wh_km, rhs=xl_k,
                        start=False, stop=False,
                    )
                    nc.tensor.matmul(
                        out=ps1[m][:], lhsT=wl_km, rhs=xh_k,
                        start=False, stop=(k == KC1 - 1),
                    )
            for m in range(MC1):
                nc.scalar.activation(
                    out=cur1[:, m * BS:(m + 1) * BS], in_=ps1[m][:],
                    func=AF.Identity, bias=b1sb[:, m:m + 1], scale=1.0,
                )

            # ---- 5 timesteps ----
            mem2_prev = None
            spk2_prev = None
            for t in range(STEPS):
                # spk1_t = H(mem1_t - 1); at t=0, mem1_0 == cur1
                src1 = cur1 if t == 0 else mem1
                nc.vector.tensor_scalar(
                    out=spk1[:], in0=src1[:], scalar1=THR, scalar2=None,
                    op0=OP.is_gt,
                )

                # fc2 into PSUM: W2.T spk1 + b2 - spk2_prev  (hi/lo 2-pass)
                ps2 = [bank(4 * (t % 2) + m, f"ps2_{t}_{m}") for m in range(MC2)]
                for m in range(MC2):
                    if t > 0:
                        nc.tensor.matmul(
                            out=ps2[m][:], lhsT=negI[:],
                            rhs=spk2_prev[:, m * BS:(m + 1) * BS],
                            start=True, stop=False,
                        )
                    nc.tensor.matmul(
                        out=ps2[m][:], lhsT=b2h[:, m * 128:(m + 1) * 128],
                        rhs=ones[:], start=(t == 0), stop=False,
                    )
                    nc.tensor.matmul(
                        out=ps2[m][:], lhsT=b2l[:, m * 128:(m + 1) * 128],
                        rhs=ones[:], start=False, stop=False,
                    )
                    for k in range(KC2):
                        s_k = spk1[:, k * BS:(k + 1) * BS]
                        w_off = k * NOUT + m * 128
                        nc.tensor.matmul(
                            out=ps2[m][:], lhsT=w2h[:, w_off:w_off + 128],
                            rhs=s_k, start=False, stop=False,
                        )
                        nc.tensor.matmul(
                            out=ps2[m][:], lhsT=w2l[:, w_off:w_off + 128],
                            rhs=s_k, start=False, stop=(k == KC2 - 1),
                        )

                # layer-1 membrane update for NEXT step (overlaps fc2 on PE):
                # mem1_{t+1} = beta*mem1_t + cur1 - spk1_t
                if t < STEPS - 1:
                    base1 = cur1 if t == 0 else mem1
                    nc.vector.scalar_tensor_tensor(
                        out=mem1[:], in0=base1[:], scalar=BETA, in1=cur1[:],
                        op0=OP.mult, op1=OP.add,
                    )
                    nc.vector.scalar_tensor_tensor(
                        out=mem1[:], in0=spk1[:], scalar=-THR, in1=mem1[:],
                        op0=OP.mult, op1=OP.add,
                    )

                # layer-2 membrane: mem2_t = beta*mem2_{t-1} + psum
                mem2_new = pool.tile([128, MC2 * BS], F32, name=f"mem2_{t}",
                                     tag="mem2", bufs=2)
                for m in range(MC2):
                    dst = mem2_new[:, m * BS:(m + 1) * BS]
                    if t == 0:
                        nc.scalar.activation(
                            out=dst, in_=ps2[m][:], func=AF.Identity,
                            bias=0.0, scale=1.0,
                        )
                    else:
                        nc.vector.scalar_tensor_tensor(
                            out=dst, in0=mem2_prev[:, m * BS:(m + 1) * BS],
                            scalar=BETA, in1=ps2[m][:],
                            op0=OP.mult, op1=OP.add,
                        )
                spk2_new = pool.tile([128, MC2 * BS], F16, name=f"spk2_{t}",
                                     tag="spk2", bufs=2)
                nc.vector.tensor_scalar(
                    out=spk2_new[:], in0=mem2_new[:], scalar1=THR, scalar2=None,
                    op0=OP.is_gt,
                )

                nc.sync.dma_start(out=d_spk[t], in_=spk2_new[:])
                nc.sync.dma_start(out=d_mem[t], in_=mem2_new[:])
                mem2_prev = mem2_new
                spk2_prev = spk2_new

    nc.compile()
    return nc


def _split16(a):
    hi = a.astype(np.float16)
    lo = (a - hi.astype(np.float32)).astype(np.float16)
    return hi, lo


def kernel(x, W1, b1, W2, b2, trace=False):
    global _prog
    if _prog is None:
        _prog = _build_program()
    nc = _prog

    x = np.asarray(x, np.float32)
    W1 = np.asarray(W1, np.float32)
    b1 = np.asarray(b1, np.float32)
    W2 = np.asarray(W2, np.float32)
    b2 = np.asarray(b2, np.float32)

    w1h, w1l = _split16(np.ascontiguousarray(W1.T))
    w2h, w2l = _split16(np.ascontiguousarray(W2.T))
    b2h, b2l = _split16(b2.reshape(1, NOUT))
    b1c = np.ascontiguousarray(b1.reshape(NH, 1).astype(np.float32))
    negI = (-np.eye(128)).astype(np.float16)

    in_maps = []
    for c in range(NCORES):
        xs = np.ascontiguousarray(x[c * BS:(c + 1) * BS].T)
        xh, xl = _split16(xs)
        in_maps.append({
            "x_hi": xh, "x_lo": xl,
            "w1_hi": w1h, "w1_lo": w1l,
            "w2_hi": w2h, "w2_lo": w2l,
            "b1": b1c, "b2_hi": b2h, "b2_lo": b2l,
            "negI": negI,
        })

    res = run_bass_kernel_spmd(nc, in_maps, list(range(NCORES)), trace=trace)

    spk_full = np.empty((STEPS, B, NOUT), np.float32)
    mem_full = np.empty((STEPS, B, NOUT), np.float32)
    for c in range(NCORES):
        r = res.results[c]
        s = r["spk_out"].reshape(STEPS, 128, MC2, BS).transpose(0, 3, 2, 1)
        m = r["mem_out"].reshape(STEPS, 128, MC2, BS).transpose(0, 3, 2, 1)
        spk_full[:, c * BS:(c + 1) * BS, :] = s.reshape(STEPS, BS, NOUT).astype(np.float32)
        mem_full[:, c * BS:(c + 1) * BS, :] = m.reshape(STEPS, BS, NOUT).astype(np.float32)

    if trace:
        return (spk_full, mem_full), res
    return spk_full, mem_full
